# revision 36
# baseline (speedup 1.0000x reference)
"""Multi-head attention kernel for 8 TRN2 NeuronCores.

Problem: B=2, S=2048, D=1024, H=16 heads, head_dim=64, fp32 I/O.

Sharding (per the tensor-parallel hint): 8 cores = 2 batches x 4 head-groups.
Core c handles batch c//4 and heads [4*(c%4), 4*(c%4)+4). Each core:
  - projects its head-slice qT/kT (feature-on-partition layout, 2 heads per
    128-partition tile) and v (natural layout, with an appended ones column),
  - computes scoresT = k @ q.T per head with K=64 row-tiled matmuls (two heads
    run concurrently in the PE array),
  - exp on ScalarE with the 1/sqrt(64) scale and the -1e9 mask folded into the
    activation's scale/bias,
  - attn@v with the [v|1] trick: the ones column makes the softmax denominator
    fall out of the same matmul stream (PSUM row 64),
  - normalizes via reciprocal + a rank-1 PE broadcast matmul,
  - computes a partial output projection over its 256 features.
Host sums the 4 partials per batch and adds the output bias.
All matmul operands are bf16 (fp32 matmul is 4x slower on the PE array);
accumulation is fp32 in PSUM and the returned partials are fp32.
"""

import numpy as np
import ml_dtypes

import concourse.mybir as mybir
import concourse.tile as tile
from concourse import bacc
from concourse.bass_utils import run_bass_kernel_spmd

BF16 = mybir.dt.bfloat16
FP32 = mybir.dt.float32

B, S, D = 2, 2048, 1024
NH, DH = 16, 64
NCORES = 8
GROUPS = 4                 # head-groups (cores per batch)
HL = NH // GROUPS          # heads per core = 4
FL = HL * DH               # features per core = 256
NPAIR = HL // 2            # head pairs per core = 2

SC = 512                   # i/s chunk (PSUM bank = 512 fp32)
JC = 128                   # j chunk (partition dim)
DCH = D // 128             # contraction chunks over embed dim = 8
N_SC = S // SC             # 4
N_JC = S // JC             # 16


def build_kernel():
    nc = bacc.Bacc("TRN2", target_bir_lowering=False, debug=False)

    xT = nc.dram_tensor("xT", [D, S], BF16, kind="ExternalInput")
    # weights arrive host-packed as one contiguous (128, 2048) block each so a
    # single large DMA loads them (24 small DMAs serialized the Sync queue)
    wq = nc.dram_tensor("wq", [128, DCH * FL], BF16, kind="ExternalInput")
    wk = nc.dram_tensor("wk", [128, DCH * FL], BF16, kind="ExternalInput")
    wv = nc.dram_tensor("wv", [128, DCH * FL], BF16, kind="ExternalInput")
    wo = nc.dram_tensor("wo", [128, 2 * D], BF16, kind="ExternalInput")
    bqc = nc.dram_tensor("bqc", [128, 2], FP32, kind="ExternalInput")
    bkc = nc.dram_tensor("bkc", [128, 2], FP32, kind="ExternalInput")
    bvb = nc.dram_tensor("bvb", [128, FL], FP32, kind="ExternalInput")
    mb = nc.dram_tensor("mb", [128, N_JC], FP32, kind="ExternalInput")
    out = nc.dram_tensor("out", [S, D], FP32, kind="ExternalOutput")

    with tile.TileContext(nc) as tc:
        with (
            tc.tile_pool(name="weights", bufs=1) as wpool,
            tc.tile_pool(name="acts", bufs=1) as apool,
            tc.tile_pool(name="exps", bufs=8) as epool,
            tc.tile_pool(name="stages", bufs=4) as spool,
            tc.tile_pool(name="smalls", bufs=3) as smpool,
            tc.tile_pool(name="scores", bufs=2, space="PSUM") as scpool,
            tc.tile_pool(name="attnout", bufs=2, space="PSUM") as aopool,
            tc.tile_pool(name="projacc", bufs=2, space="PSUM") as prpool,
        ):
            # ---- resident inputs ----
            # Two HWDGE rings drain in parallel: xT (the critical 4MB, every
            # projection psum needs all of it) streams on the Scalar ring
            # while the packed weights/biases go on the Sync ring.
            xt_all = wpool.tile([128, DCH * S], BF16, name="xt_all")
            xt = [xt_all[:, dc * S:(dc + 1) * S] for dc in range(DCH)]
            # s-chunk-major arrival: every consumer contracts over all 8
            # d-chunks, so delivering one 512-wide s-slice of ALL chunks
            # unlocks that slice's q/k/v projections after ~1MB instead of
            # the full 4MB.
            # NOTE: issue on GpSimd (SWDGE), NOT nc.scalar — HWDGE DMAs issued
            # from the Scalar sequencer occupy the ACT queue ~650ns each and
            # would delay the first exp by ~20us. Later s-chunks ride the
            # Sync ring (idle after the weights) to double the stream rate.
            for sc in range(2):
                for dc in range(DCH):
                    nc.gpsimd.dma_start(
                        out=xt[dc][:, sc * SC:(sc + 1) * SC],
                        in_=xT.ap()[dc * 128:(dc + 1) * 128, sc * SC:(sc + 1) * SC],
                    )

            wk_sb = wpool.tile([128, DCH * FL], BF16, name="wk_sb")
            nc.sync.dma_start(out=wk_sb, in_=wk.ap())
            wkt = [wk_sb[:, dc * FL:(dc + 1) * FL] for dc in range(DCH)]
            bk_sb = wpool.tile([128, 2], FP32, name="bk_sb")
            nc.sync.dma_start(out=bk_sb, in_=bkc.ap())
            wq_sb = wpool.tile([128, DCH * FL], BF16, name="wq_sb")
            nc.sync.dma_start(out=wq_sb, in_=wq.ap())
            wqt = [wq_sb[:, dc * FL:(dc + 1) * FL] for dc in range(DCH)]
            bq_sb = wpool.tile([128, 2], FP32, name="bq_sb")
            nc.sync.dma_start(out=bq_sb, in_=bqc.ap())
            mb_sb = wpool.tile([128, N_JC], FP32, name="mb_sb")
            nc.sync.dma_start(out=mb_sb, in_=mb.ap())
            wv_sb = wpool.tile([128, DCH * FL], BF16, name="wv_sb")
            nc.sync.dma_start(out=wv_sb, in_=wv.ap())
            wvt = [wv_sb[:, dc * FL:(dc + 1) * FL] for dc in range(DCH)]
            bv_sb = wpool.tile([128, FL], FP32, name="bv_sb")
            nc.sync.dma_start(out=bv_sb, in_=bvb.ap())
            wo_sb = wpool.tile([128, 2 * D], BF16, name="wo_sb")
            nc.sync.dma_start(out=wo_sb, in_=wo.ap())
            wot = [wo_sb[:, fc * D:(fc + 1) * D] for fc in range(2)]
            # xT s-chunks 2-3 on the Sync ring, queued behind the weights
            for sc in range(2, N_SC):
                for dc in range(DCH):
                    nc.sync.dma_start(
                        out=xt[dc][:, sc * SC:(sc + 1) * SC],
                        in_=xT.ap()[dc * 128:(dc + 1) * 128, sc * SC:(sc + 1) * SC],
                    )

            # ones column at partition 64 for the recip broadcast matmul
            ones65 = wpool.tile([65, 64], BF16, name="ones65")
            nc.vector.memset(ones65[64:65, :], 1.0)
            # warm the ScalarE Exp table set while DMAs stream (saves the
            # ~2.7us ACT_TABLE_LOAD from delaying the first real exp)
            warm = smpool.tile([1, 4], FP32, name="warm", tag="warm")
            nc.vector.memset(warm, 1.0)
            nc.scalar.activation(warm, warm, mybir.ActivationFunctionType.Exp)

            # ---- persistent activations ----
            # qT/kT: tile p holds features [128p,128p+128) = heads 2p,2p+1
            qt = [apool.tile([128, S], BF16, name=f"qt{p}") for p in range(2)]
            kt = [apool.tile([128, S], BF16, name=f"kt{p}") for p in range(2)]
            # v natural: tile sc = rows [128sc,128sc+128), layout (128, 4 heads, 65)
            vt = [apool.tile([128, HL, 65], BF16, name=f"vt{sc}") for sc in range(N_JC)]
            # normalized attention output, transposed: (features, S)
            at = [apool.tile([128, S], BF16, name=f"at{p}") for p in range(2)]

            qk_open = {}  # key -> open psum accumulation tile

            def qk_half(dst, w_tiles, bias_sb, sc, fc, half):
                """Half of a qT/kT projection s-chunk (4 of 8 d-accumulation
                matmuls, ~0.9us of PE) so drip slots stay small. The psum
                group stays open between halves."""
                key = (id(dst), sc)
                if half == 0:
                    ps = prpool.tile([128, SC], FP32, name="ps", tag="ps")
                    qk_open[key] = ps
                else:
                    ps = qk_open.pop(key)
                for dc in range(half * 4, half * 4 + 4):
                    nc.tensor.matmul(
                        ps,
                        lhsT=w_tiles[dc][:, fc * 128:(fc + 1) * 128],
                        rhs=xt[dc][:, sc * SC:(sc + 1) * SC],
                        start=(dc == 0),
                        stop=(dc == DCH - 1),
                    )
                if half == 1:
                    nc.vector.tensor_scalar_add(
                        dst[:, sc * SC:(sc + 1) * SC], ps, bias_sb[:, fc:fc + 1]
                    )

            def qk_full(dst, w_tiles, bias_sb, sc, fc):
                qk_half(dst, w_tiles, bias_sb, sc, fc, 0)
                qk_half(dst, w_tiles, bias_sb, sc, fc, 1)

            def v_proj(sc, pair):
                """v rows [128sc,+128) for one head-pair (N=128, ~0.9us)."""
                ps = prpool.tile([128, 128], FP32, name="ps", tag="ps")
                for dc in range(DCH):
                    nc.tensor.matmul(
                        ps,
                        lhsT=xt[dc][:, sc * JC:(sc + 1) * JC],
                        rhs=wvt[dc][:, pair * 128:(pair + 1) * 128],
                        start=(dc == 0),
                        stop=(dc == DCH - 1),
                    )
                nc.vector.tensor_add(
                    vt[sc][:, 2 * pair:2 * pair + 2, 0:64],
                    ps.rearrange("p (h d) -> p h d", h=2),
                    bv_sb[:, pair * 128:(pair + 1) * 128].rearrange("p (h d) -> p h d", h=2),
                )
                if pair == 0:
                    nc.vector.memset(vt[sc][:, :, 64:65], 1.0)

            def attention(pair, per_jc_hook=None, per_ic_hook=None):
                """Full attention for heads (2*pair, 2*pair+1).

                per_jc_hook(ic, jc) / per_ic_hook(ic) emit extra work
                interleaved into the PE stream to keep it dense."""
                for ic in range(N_SC):
                    i_sl = slice(ic * SC, (ic + 1) * SC)
                    outA = aopool.tile([65, SC], FP32, name="outA", tag="ao")
                    outB = aopool.tile([65, SC], FP32, name="outB", tag="ao")
                    for jc in range(N_JC):
                        sc_ps = scpool.tile([128, 2 * SC], FP32, name="sc_ps")
                        # scoresT = k @ q.T, two heads row-tiled (K=64 each)
                        nc.tensor.matmul(
                            sc_ps[:, 0:SC],
                            lhsT=kt[pair][0:64, jc * JC:(jc + 1) * JC],
                            rhs=qt[pair][0:64, i_sl],
                        )
                        nc.tensor.matmul(
                            sc_ps[:, SC:2 * SC],
                            lhsT=kt[pair][64:128, jc * JC:(jc + 1) * JC],
                            rhs=qt[pair][64:128, i_sl],
                        )
                        ex = epool.tile([128, 2 * SC], BF16, name="ex")
                        nc.scalar.activation(
                            ex, sc_ps, mybir.ActivationFunctionType.Exp,
                            bias=mb_sb[:, jc:jc + 1], scale=1.0 / np.sqrt(DH),
                        )
                        if per_jc_hook is not None:
                            per_jc_hook(ic, jc)
                        nc.tensor.matmul(
                            outA, lhsT=vt[jc][:, 2 * pair, :], rhs=ex[:, 0:SC],
                            start=(jc == 0), stop=(jc == N_JC - 1),
                        )
                        nc.tensor.matmul(
                            outB, lhsT=vt[jc][:, 2 * pair + 1, :], rhs=ex[:, SC:2 * SC],
                            start=(jc == 0), stop=(jc == N_JC - 1),
                        )
                    # normalize: rows 0..63 are attn@v, row 64 is sum(exp).
                    # Copy PSUM->SBUF right away to free the accumulator banks.
                    for half, ps_o in ((0, outA), (1, outB)):
                        osb = smpool.tile([65, SC], FP32, name="osb", tag="osb")
                        nc.vector.tensor_copy(osb, ps_o)
                        # reciprocal cost scales with free-size per lane:
                        # reshape Z (1,512) -> (64,8) via DMA so the recip
                        # runs 8 elems/lane instead of 512, then DMA-cast
                        # back to a bf16 row for the broadcast matmul.
                        zsp = smpool.tile([64, SC // 64], FP32, name="zsp", tag="zsp")
                        nc.gpsimd.dma_start(out=zsp, in_=osb[64:65, :])
                        rsp = smpool.tile([64, SC // 64], FP32, name="rsp", tag="rsp")
                        nc.vector.reciprocal(rsp, zsp)
                        rec_bf = smpool.tile([65, SC], BF16, name="rec_bf", tag="recbf")
                        nc.gpsimd.dma_start(out=rec_bf[64:65, :], in_=rsp)
                        bc = prpool.tile([64, SC], FP32, name="bc", tag="ps")
                        nc.tensor.matmul(bc, lhsT=ones65[64:65, :], rhs=rec_bf[64:65, :])
                        if half == 0:
                            nc.vector.tensor_mul(at[pair][0:64, i_sl], osb[0:64, :], bc)
                        else:
                            stg = smpool.tile([64, SC], BF16, name="stg", tag="stg")
                            nc.vector.tensor_mul(stg, osb[0:64, :], bc)
                            # shift to partitions 64..127 (DVE can't cross lanes)
                            nc.sync.dma_start(out=at[pair][64:128, i_sl], in_=stg)
                    if per_ic_hook is not None:
                        per_ic_hook(ic)

            def out_proj_chunk(ic, ec, ss):
                """One (128 s, 512 e) chunk of the partial output projection."""
                srow = ic * SC + ss * 128
                po = prpool.tile([128, SC], FP32, name="po", tag="ps")
                for fc in range(2):
                    nc.tensor.matmul(
                        po,
                        lhsT=at[fc][:, srow:srow + 128],
                        rhs=wot[fc][:, ec * SC:(ec + 1) * SC],
                        start=(fc == 0),
                        stop=(fc == 1),
                    )
                stg = spool.tile([128, SC], FP32, name="ostg")
                nc.vector.tensor_copy(stg, po)
                nc.sync.dma_start(
                    out=out.ap()[srow:srow + 128, ec * SC:(ec + 1) * SC],
                    in_=stg,
                )

            # ---- emission order (drives scheduling priority and the
            # per-engine instruction streams; engines execute in order) ----
            #
            # 8 attention blocks (pair, ic). All projection / out-proj work
            # beyond a minimal prefix is dripped into the jc loops at <=1us
            # per slot with deadlines, so the PE stream per jc stays under
            # the ~1.15us exp pace and ScalarE never starves:
            #   block 0 (p0,ic0): vt pair-0 streaming (vt[j] by jc=j) and
            #                     k0 halves (s-chunk s by jc=4s)
            #   blocks 1-3:       pair-0 q leftovers, pair-1 v, pair-1 q/k
            #   blocks 4-7:       previous ic's out_proj chunks
            K0, Q0, K1, Q1 = (kt[0], wkt, bk_sb, 0), (qt[0], wqt, bq_sb, 0), \
                             (kt[1], wkt, bk_sb, 1), (qt[1], wqt, bq_sb, 1)

            def qk_thunk(args, scn, half):
                dst, w, b, fc = args
                return lambda: qk_half(dst, w, b, scn, fc, half)

            sched = {b: {} for b in range(8)}

            def put(b, jc, thunk):
                sched[b].setdefault(jc, []).append(thunk)

            # block 0: v pair-0 streaming + k0 + q0 sc1
            for j in range(1, N_JC):
                put(0, j - 1, lambda j=j: v_proj(j, 0))
            put(0, 1, qk_thunk(K0, 1, 0)); put(0, 2, qk_thunk(K0, 1, 1))
            put(0, 5, qk_thunk(K0, 2, 0)); put(0, 6, qk_thunk(K0, 2, 1))
            put(0, 9, qk_thunk(K0, 3, 0)); put(0, 10, qk_thunk(K0, 3, 1))
            put(0, 12, qk_thunk(Q0, 1, 0)); put(0, 13, qk_thunk(Q0, 1, 1))
            # blocks 1-7: drips start at jc>=3 so block starts stay clean
            # (the first couple of scores after a boundary refill the psum
            # pipeline; extra PE work there directly stalls ScalarE).
            # q1 s-chunk i is only needed from block 4+i on, so it can ride
            # blocks 4-6; k1 is needed in full by every pair-1 block.
            # each block's first item sits at the PREVIOUS block's jc15: it
            # fills the boundary bubble (scores-psum refill) with PE work so
            # HAM stays warm, and runs strictly earlier so deadlines hold
            put(0, 15, qk_thunk(Q0, 2, 0)); put(1, 3, qk_thunk(Q0, 2, 1))
            put(1, 5, qk_thunk(Q0, 3, 0)); put(1, 6, qk_thunk(Q0, 3, 1))
            for i, j in enumerate(range(0, 5)):
                put(1, 7 + i, lambda j=j: v_proj(j, 1))
            put(1, 15, lambda: v_proj(5, 1))
            # block 2 was over budget (10 v-chunks > the per-block jc slack);
            # spread 6 chunks on alternating jc's and push the rest to blocks
            # 3-4 (vt[j] pair-1 is only needed at block 4, jc=j)
            for i, j in enumerate(range(6, 12)):
                put(2, 3 + 2 * i, lambda j=j: v_proj(j, 1))
            put(2, 15, qk_thunk(K1, 0, 0)); put(3, 3, qk_thunk(K1, 0, 1))
            put(3, 5, qk_thunk(K1, 1, 0)); put(3, 6, qk_thunk(K1, 1, 1))
            put(3, 7, qk_thunk(K1, 2, 0)); put(3, 8, qk_thunk(K1, 2, 1))
            put(3, 9, qk_thunk(K1, 3, 0)); put(3, 10, qk_thunk(K1, 3, 1))
            put(3, 11, qk_thunk(Q1, 0, 0)); put(3, 12, qk_thunk(Q1, 0, 1))
            put(3, 13, lambda: v_proj(12, 1)); put(3, 14, lambda: v_proj(13, 1))
            put(3, 15, qk_thunk(Q1, 1, 0)); put(4, 3, qk_thunk(Q1, 1, 1))
            put(4, 5, lambda: v_proj(14, 1)); put(4, 7, lambda: v_proj(15, 1))
            put(4, 15, qk_thunk(Q1, 2, 0)); put(5, 3, qk_thunk(Q1, 2, 1))
            put(5, 15, qk_thunk(Q1, 3, 0)); put(6, 3, qk_thunk(Q1, 3, 1))
            # blocks 5-7: drip previous ic's out_proj (8 chunks each)
            for b in range(5, 8):
                ic_prev = b - 5
                idx = 0
                for ec in range(2):
                    for ss in range(SC // 128):
                        put(b, 5 + idx, lambda ic=ic_prev, ec=ec, ss=ss:
                            out_proj_chunk(ic, ec, ss))
                        idx += 1

            def hook(block):
                def _h(ic, jc):
                    for thunk in sched[block].get(jc, []):
                        thunk()
                return _h

            # minimal prefix: k0/q0 s-chunk 0 and vt[0] pair 0
            qk_full(kt[0], wkt, bk_sb, 0, 0)
            qk_full(qt[0], wqt, bq_sb, 0, 0)
            v_proj(0, 0)

            attention(0, per_jc_hook=lambda ic, jc: hook(ic)(ic, jc))
            attention(1, per_jc_hook=lambda ic, jc: hook(4 + ic)(ic, jc))
            # final ic's output projection (tail)
            for ec in range(2):
                for ss in range(SC // 128):
                    out_proj_chunk(N_SC - 1, ec, ss)

    nc.compile()
    return nc


_NC_CACHE = None


def _get_nc():
    global _NC_CACHE
    if _NC_CACHE is None:
        _NC_CACHE = build_kernel()
    return _NC_CACHE


def make_in_maps(inputs):
    x = np.asarray(inputs["x"], dtype=np.float32)
    mask = np.asarray(inputs["mask"])
    Wq = np.asarray(inputs["Wq"], dtype=np.float32)
    bq = np.asarray(inputs["bq"], dtype=np.float32)
    Wk = np.asarray(inputs["Wk"], dtype=np.float32)
    bk = np.asarray(inputs["bk"], dtype=np.float32)
    Wv = np.asarray(inputs["Wv"], dtype=np.float32)
    bv = np.asarray(inputs["bv"], dtype=np.float32)
    Wo = np.asarray(inputs["Wo"], dtype=np.float32)

    bf = ml_dtypes.bfloat16

    def pack_dxf(wT):  # (1024, FL) -> (128, 8*FL): d-chunks side by side
        return np.ascontiguousarray(
            wT.reshape(DCH, 128, FL).transpose(1, 0, 2).reshape(128, DCH * FL)
        )

    def pack_fxe(woT):  # (256, D) -> (128, 2*D): f-chunks side by side
        return np.ascontiguousarray(
            woT.reshape(2, 128, D).transpose(1, 0, 2).reshape(128, 2 * D)
        )

    in_maps = []
    for c in range(NCORES):
        b = c // GROUPS
        g = c % GROUPS
        fs, fe = g * FL, (g + 1) * FL
        in_maps.append({
            "xT": np.ascontiguousarray(x[b].T).astype(bf),
            "wq": pack_dxf(Wq[fs:fe, :].T.astype(bf)),
            "wk": pack_dxf(Wk[fs:fe, :].T.astype(bf)),
            "wv": pack_dxf(Wv[fs:fe, :].T.astype(bf)),
            "wo": pack_fxe(Wo[:, fs:fe].T.astype(bf)),
            "bqc": np.ascontiguousarray(bq[fs:fe].reshape(2, 128).T),
            "bkc": np.ascontiguousarray(bk[fs:fe].reshape(2, 128).T),
            "bvb": np.tile(bv[fs:fe], (128, 1)).astype(np.float32),
            "mb": np.ascontiguousarray(
                np.where(mask[b] == 0, np.float32(-1e9), np.float32(0.0))
                .astype(np.float32).reshape(N_JC, 128).T
            ),
        })
    return in_maps


def kernel(x, mask, Wq, bq, Wk, bk, Wv, bv, Wo, bo):
    bo = np.asarray(bo, dtype=np.float32)
    nc = _get_nc()
    in_maps = make_in_maps(dict(x=x, mask=mask, Wq=Wq, bq=bq, Wk=Wk, bk=bk,
                                Wv=Wv, bv=bv, Wo=Wo, bo=bo))
    res = run_bass_kernel_spmd(nc, in_maps, core_ids=list(range(NCORES)))
    parts = [np.asarray(r["out"], dtype=np.float32) for r in res.results]
    full = np.empty((B, S, D), dtype=np.float32)
    for b in range(B):
        acc = parts[b * GROUPS].copy()
        for g in range(1, GROUPS):
            acc += parts[b * GROUPS + g]
        full[b] = acc + bo[None, :]
    return full


# revision 37
# speedup vs baseline: 1.0031x; 1.0031x over previous
"""Multi-head attention kernel for 8 TRN2 NeuronCores.

Problem: B=2, S=2048, D=1024, H=16 heads, head_dim=64, fp32 I/O.

Sharding (per the tensor-parallel hint): 8 cores = 2 batches x 4 head-groups.
Core c handles batch c//4 and heads [4*(c%4), 4*(c%4)+4). Each core:
  - projects its head-slice qT/kT (feature-on-partition layout, 2 heads per
    128-partition tile) and v (natural layout, with an appended ones column),
  - computes scoresT = k @ q.T per head with K=64 row-tiled matmuls (two heads
    run concurrently in the PE array),
  - exp on ScalarE with the 1/sqrt(64) scale and the -1e9 mask folded into the
    activation's scale/bias,
  - attn@v with the [v|1] trick: the ones column makes the softmax denominator
    fall out of the same matmul stream (PSUM row 64),
  - normalizes via reciprocal + a rank-1 PE broadcast matmul,
  - computes a partial output projection over its 256 features.
Host sums the 4 partials per batch and adds the output bias.
All matmul operands are bf16 (fp32 matmul is 4x slower on the PE array);
accumulation is fp32 in PSUM and the returned partials are fp32.
"""

import numpy as np
import ml_dtypes

import concourse.mybir as mybir
import concourse.tile as tile
from concourse import bacc
from concourse.bass_utils import run_bass_kernel_spmd

BF16 = mybir.dt.bfloat16
FP32 = mybir.dt.float32

B, S, D = 2, 2048, 1024
NH, DH = 16, 64
NCORES = 8
GROUPS = 4                 # head-groups (cores per batch)
HL = NH // GROUPS          # heads per core = 4
FL = HL * DH               # features per core = 256
NPAIR = HL // 2            # head pairs per core = 2

SC = 512                   # i/s chunk (PSUM bank = 512 fp32)
JC = 128                   # j chunk (partition dim)
DCH = D // 128             # contraction chunks over embed dim = 8
N_SC = S // SC             # 4
N_JC = S // JC             # 16


def build_kernel():
    nc = bacc.Bacc("TRN2", target_bir_lowering=False, debug=False)

    xT = nc.dram_tensor("xT", [D, S], BF16, kind="ExternalInput")
    # weights arrive host-packed as one contiguous (128, 2048) block each so a
    # single large DMA loads them (24 small DMAs serialized the Sync queue)
    wq = nc.dram_tensor("wq", [128, DCH * FL], BF16, kind="ExternalInput")
    wk = nc.dram_tensor("wk", [128, DCH * FL], BF16, kind="ExternalInput")
    wv = nc.dram_tensor("wv", [128, DCH * FL], BF16, kind="ExternalInput")
    wo = nc.dram_tensor("wo", [128, 2 * D], BF16, kind="ExternalInput")
    bqc = nc.dram_tensor("bqc", [128, 2], FP32, kind="ExternalInput")
    bkc = nc.dram_tensor("bkc", [128, 2], FP32, kind="ExternalInput")
    bvb = nc.dram_tensor("bvb", [128, FL], FP32, kind="ExternalInput")
    mb = nc.dram_tensor("mb", [128, N_JC], FP32, kind="ExternalInput")
    out = nc.dram_tensor("out", [S, D], FP32, kind="ExternalOutput")

    with tile.TileContext(nc) as tc:
        with (
            tc.tile_pool(name="weights", bufs=1) as wpool,
            tc.tile_pool(name="acts", bufs=1) as apool,
            tc.tile_pool(name="exps", bufs=6) as epool,
            tc.tile_pool(name="stages", bufs=4) as spool,
            tc.tile_pool(name="smalls", bufs=3) as smpool,
            tc.tile_pool(name="scores", bufs=2, space="PSUM") as scpool,
            tc.tile_pool(name="attnout", bufs=2, space="PSUM") as aopool,
            tc.tile_pool(name="projacc", bufs=2, space="PSUM") as prpool,
        ):
            # ---- resident inputs ----
            # Two HWDGE rings drain in parallel: xT (the critical 4MB, every
            # projection psum needs all of it) streams on the Scalar ring
            # while the packed weights/biases go on the Sync ring.
            xt_all = wpool.tile([128, DCH * S], BF16, name="xt_all")
            xt = [xt_all[:, dc * S:(dc + 1) * S] for dc in range(DCH)]
            # s-chunk-major arrival: every consumer contracts over all 8
            # d-chunks, so delivering one 512-wide s-slice of ALL chunks
            # unlocks that slice's q/k/v projections after ~1MB instead of
            # the full 4MB.
            # NOTE: issue on GpSimd (SWDGE), NOT nc.scalar — HWDGE DMAs issued
            # from the Scalar sequencer occupy the ACT queue ~650ns each and
            # would delay the first exp by ~20us. Later s-chunks ride the
            # Sync ring (idle after the weights) to double the stream rate.
            for sc in range(2):
                for dc in range(DCH):
                    nc.gpsimd.dma_start(
                        out=xt[dc][:, sc * SC:(sc + 1) * SC],
                        in_=xT.ap()[dc * 128:(dc + 1) * 128, sc * SC:(sc + 1) * SC],
                    )

            wk_sb = wpool.tile([128, DCH * FL], BF16, name="wk_sb")
            nc.sync.dma_start(out=wk_sb, in_=wk.ap())
            wkt = [wk_sb[:, dc * FL:(dc + 1) * FL] for dc in range(DCH)]
            bk_sb = wpool.tile([128, 2], FP32, name="bk_sb")
            nc.sync.dma_start(out=bk_sb, in_=bkc.ap())
            wq_sb = wpool.tile([128, DCH * FL], BF16, name="wq_sb")
            nc.sync.dma_start(out=wq_sb, in_=wq.ap())
            wqt = [wq_sb[:, dc * FL:(dc + 1) * FL] for dc in range(DCH)]
            bq_sb = wpool.tile([128, 2], FP32, name="bq_sb")
            nc.sync.dma_start(out=bq_sb, in_=bqc.ap())
            mb_sb = wpool.tile([128, N_JC], FP32, name="mb_sb")
            nc.sync.dma_start(out=mb_sb, in_=mb.ap())
            wv_sb = wpool.tile([128, DCH * FL], BF16, name="wv_sb")
            nc.sync.dma_start(out=wv_sb, in_=wv.ap())
            wvt = [wv_sb[:, dc * FL:(dc + 1) * FL] for dc in range(DCH)]
            bv_sb = wpool.tile([128, FL], FP32, name="bv_sb")
            nc.sync.dma_start(out=bv_sb, in_=bvb.ap())
            wo_sb = wpool.tile([128, 2 * D], BF16, name="wo_sb")
            nc.sync.dma_start(out=wo_sb, in_=wo.ap())
            wot = [wo_sb[:, fc * D:(fc + 1) * D] for fc in range(2)]
            # xT s-chunks 2-3 on the Sync ring, queued behind the weights
            for sc in range(2, N_SC):
                for dc in range(DCH):
                    nc.sync.dma_start(
                        out=xt[dc][:, sc * SC:(sc + 1) * SC],
                        in_=xT.ap()[dc * 128:(dc + 1) * 128, sc * SC:(sc + 1) * SC],
                    )

            # ones column at partition 64 for the recip broadcast matmul
            ones65 = wpool.tile([65, 64], BF16, name="ones65")
            nc.vector.memset(ones65[64:65, :], 1.0)
            # warm the ScalarE Exp table set while DMAs stream (saves the
            # ~2.7us ACT_TABLE_LOAD from delaying the first real exp)
            warm = smpool.tile([1, 4], FP32, name="warm", tag="warm")
            nc.vector.memset(warm, 1.0)
            nc.scalar.activation(warm, warm, mybir.ActivationFunctionType.Exp)

            # ---- persistent activations ----
            # qT/kT: tile p holds features [128p,128p+128) = heads 2p,2p+1
            qt = [apool.tile([128, S], BF16, name=f"qt{p}") for p in range(2)]
            kt = [apool.tile([128, S], BF16, name=f"kt{p}") for p in range(2)]
            # v natural: tile sc = rows [128sc,128sc+128), layout (128, 4 heads, 65)
            vt = [apool.tile([128, HL, 65], BF16, name=f"vt{sc}") for sc in range(N_JC)]
            # normalized attention output, transposed: (features, S)
            at = [apool.tile([128, S], BF16, name=f"at{p}") for p in range(2)]

            qk_open = {}  # key -> open psum accumulation tile

            def qk_half(dst, w_tiles, bias_sb, sc, fc, half):
                """Half of a qT/kT projection s-chunk (4 of 8 d-accumulation
                matmuls, ~0.9us of PE) so drip slots stay small. The psum
                group stays open between halves."""
                key = (id(dst), sc)
                if half == 0:
                    ps = prpool.tile([128, SC], FP32, name="ps", tag="ps")
                    qk_open[key] = ps
                else:
                    ps = qk_open.pop(key)
                for dc in range(half * 4, half * 4 + 4):
                    nc.tensor.matmul(
                        ps,
                        lhsT=w_tiles[dc][:, fc * 128:(fc + 1) * 128],
                        rhs=xt[dc][:, sc * SC:(sc + 1) * SC],
                        start=(dc == 0),
                        stop=(dc == DCH - 1),
                    )
                if half == 1:
                    nc.vector.tensor_scalar_add(
                        dst[:, sc * SC:(sc + 1) * SC], ps, bias_sb[:, fc:fc + 1]
                    )

            def qk_full(dst, w_tiles, bias_sb, sc, fc):
                qk_half(dst, w_tiles, bias_sb, sc, fc, 0)
                qk_half(dst, w_tiles, bias_sb, sc, fc, 1)

            def v_proj(sc, pair):
                """v rows [128sc,+128) for one head-pair (N=128, ~0.9us)."""
                ps = prpool.tile([128, 128], FP32, name="ps", tag="ps")
                for dc in range(DCH):
                    nc.tensor.matmul(
                        ps,
                        lhsT=xt[dc][:, sc * JC:(sc + 1) * JC],
                        rhs=wvt[dc][:, pair * 128:(pair + 1) * 128],
                        start=(dc == 0),
                        stop=(dc == DCH - 1),
                    )
                nc.vector.tensor_add(
                    vt[sc][:, 2 * pair:2 * pair + 2, 0:64],
                    ps.rearrange("p (h d) -> p h d", h=2),
                    bv_sb[:, pair * 128:(pair + 1) * 128].rearrange("p (h d) -> p h d", h=2),
                )
                if pair == 0:
                    nc.vector.memset(vt[sc][:, :, 64:65], 1.0)

            def attention(pair, per_jc_hook=None, per_ic_hook=None):
                """Full attention for heads (2*pair, 2*pair+1).

                per_jc_hook(ic, jc) / per_ic_hook(ic) emit extra work
                interleaved into the PE stream to keep it dense."""
                for ic in range(N_SC):
                    i_sl = slice(ic * SC, (ic + 1) * SC)
                    outA = aopool.tile([65, SC], FP32, name="outA", tag="ao")
                    outB = aopool.tile([65, SC], FP32, name="outB", tag="ao")
                    for jc in range(N_JC):
                        sc_ps = scpool.tile([128, 2 * SC], FP32, name="sc_ps")
                        # scoresT = k @ q.T, two heads row-tiled (K=64 each)
                        nc.tensor.matmul(
                            sc_ps[:, 0:SC],
                            lhsT=kt[pair][0:64, jc * JC:(jc + 1) * JC],
                            rhs=qt[pair][0:64, i_sl],
                        )
                        nc.tensor.matmul(
                            sc_ps[:, SC:2 * SC],
                            lhsT=kt[pair][64:128, jc * JC:(jc + 1) * JC],
                            rhs=qt[pair][64:128, i_sl],
                        )
                        ex = epool.tile([128, 2 * SC], BF16, name="ex")
                        nc.scalar.activation(
                            ex, sc_ps, mybir.ActivationFunctionType.Exp,
                            bias=mb_sb[:, jc:jc + 1], scale=1.0 / np.sqrt(DH),
                        )
                        if per_jc_hook is not None:
                            per_jc_hook(ic, jc)
                        nc.tensor.matmul(
                            outA, lhsT=vt[jc][:, 2 * pair, :], rhs=ex[:, 0:SC],
                            start=(jc == 0), stop=(jc == N_JC - 1),
                        )
                        nc.tensor.matmul(
                            outB, lhsT=vt[jc][:, 2 * pair + 1, :], rhs=ex[:, SC:2 * SC],
                            start=(jc == 0), stop=(jc == N_JC - 1),
                        )
                    # normalize: rows 0..63 are attn@v, row 64 is sum(exp).
                    # Copy PSUM->SBUF right away to free the accumulator banks.
                    for half, ps_o in ((0, outA), (1, outB)):
                        osb = smpool.tile([65, SC], FP32, name="osb", tag="osb")
                        nc.vector.tensor_copy(osb, ps_o)
                        # reciprocal cost scales with free-size per lane:
                        # reshape Z (1,512) -> (64,8) via DMA so the recip
                        # runs 8 elems/lane instead of 512, then DMA-cast
                        # back to a bf16 row for the broadcast matmul.
                        zsp = smpool.tile([64, SC // 64], FP32, name="zsp", tag="zsp")
                        nc.gpsimd.dma_start(out=zsp, in_=osb[64:65, :])
                        rsp = smpool.tile([64, SC // 64], FP32, name="rsp", tag="rsp")
                        nc.vector.reciprocal(rsp, zsp)
                        rec_bf = smpool.tile([65, SC], BF16, name="rec_bf", tag="recbf")
                        nc.gpsimd.dma_start(out=rec_bf[64:65, :], in_=rsp)
                        bc = prpool.tile([64, SC], FP32, name="bc", tag="ps")
                        nc.tensor.matmul(bc, lhsT=ones65[64:65, :], rhs=rec_bf[64:65, :])
                        if half == 0:
                            nc.vector.tensor_mul(at[pair][0:64, i_sl], osb[0:64, :], bc)
                        else:
                            stg = smpool.tile([64, SC], BF16, name="stg", tag="stg")
                            nc.vector.tensor_mul(stg, osb[0:64, :], bc)
                            # shift to partitions 64..127 (DVE can't cross lanes)
                            nc.sync.dma_start(out=at[pair][64:128, i_sl], in_=stg)
                    if per_ic_hook is not None:
                        per_ic_hook(ic)

            def out_proj_chunk(ic, ec, ss):
                """One (128 s, 512 e) chunk of the partial output projection."""
                srow = ic * SC + ss * 128
                po = prpool.tile([128, SC], FP32, name="po", tag="ps")
                for fc in range(2):
                    nc.tensor.matmul(
                        po,
                        lhsT=at[fc][:, srow:srow + 128],
                        rhs=wot[fc][:, ec * SC:(ec + 1) * SC],
                        start=(fc == 0),
                        stop=(fc == 1),
                    )
                stg = spool.tile([128, SC], FP32, name="ostg")
                nc.vector.tensor_copy(stg, po)
                nc.sync.dma_start(
                    out=out.ap()[srow:srow + 128, ec * SC:(ec + 1) * SC],
                    in_=stg,
                )

            # ---- emission order (drives scheduling priority and the
            # per-engine instruction streams; engines execute in order) ----
            #
            # 8 attention blocks (pair, ic). All projection / out-proj work
            # beyond a minimal prefix is dripped into the jc loops at <=1us
            # per slot with deadlines, so the PE stream per jc stays under
            # the ~1.15us exp pace and ScalarE never starves:
            #   block 0 (p0,ic0): vt pair-0 streaming (vt[j] by jc=j) and
            #                     k0 halves (s-chunk s by jc=4s)
            #   blocks 1-3:       pair-0 q leftovers, pair-1 v, pair-1 q/k
            #   blocks 4-7:       previous ic's out_proj chunks
            K0, Q0, K1, Q1 = (kt[0], wkt, bk_sb, 0), (qt[0], wqt, bq_sb, 0), \
                             (kt[1], wkt, bk_sb, 1), (qt[1], wqt, bq_sb, 1)

            def qk_thunk(args, scn, half):
                dst, w, b, fc = args
                return lambda: qk_half(dst, w, b, scn, fc, half)

            sched = {b: {} for b in range(8)}

            def put(b, jc, thunk):
                sched[b].setdefault(jc, []).append(thunk)

            # block 0: v pair-0 streaming + k0 + q0 sc1
            for j in range(1, N_JC):
                put(0, j - 1, lambda j=j: v_proj(j, 0))
            put(0, 1, qk_thunk(K0, 1, 0)); put(0, 2, qk_thunk(K0, 1, 1))
            put(0, 5, qk_thunk(K0, 2, 0)); put(0, 6, qk_thunk(K0, 2, 1))
            put(0, 9, qk_thunk(K0, 3, 0)); put(0, 10, qk_thunk(K0, 3, 1))
            put(0, 12, qk_thunk(Q0, 1, 0)); put(0, 13, qk_thunk(Q0, 1, 1))
            # blocks 1-7: drips start at jc>=3 so block starts stay clean
            # (the first couple of scores after a boundary refill the psum
            # pipeline; extra PE work there directly stalls ScalarE).
            # q1 s-chunk i is only needed from block 4+i on, so it can ride
            # blocks 4-6; k1 is needed in full by every pair-1 block.
            # each block's first item sits at the PREVIOUS block's jc15: it
            # fills the boundary bubble (scores-psum refill) with PE work so
            # HAM stays warm, and runs strictly earlier so deadlines hold
            put(0, 15, qk_thunk(Q0, 2, 0)); put(1, 3, qk_thunk(Q0, 2, 1))
            put(1, 5, qk_thunk(Q0, 3, 0)); put(1, 6, qk_thunk(Q0, 3, 1))
            for i, j in enumerate(range(0, 5)):
                put(1, 7 + i, lambda j=j: v_proj(j, 1))
            put(1, 15, lambda: v_proj(5, 1))
            # block 2 was over budget (10 v-chunks > the per-block jc slack);
            # spread 6 chunks on alternating jc's and push the rest to blocks
            # 3-4 (vt[j] pair-1 is only needed at block 4, jc=j)
            for i, j in enumerate(range(6, 12)):
                put(2, 3 + 2 * i, lambda j=j: v_proj(j, 1))
            put(2, 15, qk_thunk(K1, 0, 0)); put(3, 3, qk_thunk(K1, 0, 1))
            put(3, 5, qk_thunk(K1, 1, 0)); put(3, 6, qk_thunk(K1, 1, 1))
            put(3, 7, qk_thunk(K1, 2, 0)); put(3, 8, qk_thunk(K1, 2, 1))
            put(3, 9, qk_thunk(K1, 3, 0)); put(3, 10, qk_thunk(K1, 3, 1))
            put(3, 11, qk_thunk(Q1, 0, 0)); put(3, 12, qk_thunk(Q1, 0, 1))
            put(3, 13, lambda: v_proj(12, 1)); put(3, 14, lambda: v_proj(13, 1))
            put(3, 15, qk_thunk(Q1, 1, 0)); put(4, 3, qk_thunk(Q1, 1, 1))
            put(4, 5, lambda: v_proj(14, 1)); put(4, 7, lambda: v_proj(15, 1))
            put(4, 15, qk_thunk(Q1, 2, 0)); put(5, 3, qk_thunk(Q1, 2, 1))
            put(5, 15, qk_thunk(Q1, 3, 0)); put(6, 3, qk_thunk(Q1, 3, 1))
            # blocks 5-7: drip previous ic's out_proj (8 chunks each)
            for b in range(5, 8):
                ic_prev = b - 5
                idx = 0
                for ec in range(2):
                    for ss in range(SC // 128):
                        put(b, 5 + idx, lambda ic=ic_prev, ec=ec, ss=ss:
                            out_proj_chunk(ic, ec, ss))
                        idx += 1

            def hook(block):
                def _h(ic, jc):
                    for thunk in sched[block].get(jc, []):
                        thunk()
                return _h

            # minimal prefix: k0/q0 s-chunk 0 and vt[0] pair 0
            qk_full(kt[0], wkt, bk_sb, 0, 0)
            qk_full(qt[0], wqt, bq_sb, 0, 0)
            v_proj(0, 0)

            attention(0, per_jc_hook=lambda ic, jc: hook(ic)(ic, jc))
            attention(1, per_jc_hook=lambda ic, jc: hook(4 + ic)(ic, jc))
            # final ic's output projection (tail)
            for ec in range(2):
                for ss in range(SC // 128):
                    out_proj_chunk(N_SC - 1, ec, ss)

    nc.compile()
    return nc


_NC_CACHE = None


def _get_nc():
    global _NC_CACHE
    if _NC_CACHE is None:
        _NC_CACHE = build_kernel()
    return _NC_CACHE


def make_in_maps(inputs):
    x = np.asarray(inputs["x"], dtype=np.float32)
    mask = np.asarray(inputs["mask"])
    Wq = np.asarray(inputs["Wq"], dtype=np.float32)
    bq = np.asarray(inputs["bq"], dtype=np.float32)
    Wk = np.asarray(inputs["Wk"], dtype=np.float32)
    bk = np.asarray(inputs["bk"], dtype=np.float32)
    Wv = np.asarray(inputs["Wv"], dtype=np.float32)
    bv = np.asarray(inputs["bv"], dtype=np.float32)
    Wo = np.asarray(inputs["Wo"], dtype=np.float32)

    bf = ml_dtypes.bfloat16

    def pack_dxf(wT):  # (1024, FL) -> (128, 8*FL): d-chunks side by side
        return np.ascontiguousarray(
            wT.reshape(DCH, 128, FL).transpose(1, 0, 2).reshape(128, DCH * FL)
        )

    def pack_fxe(woT):  # (256, D) -> (128, 2*D): f-chunks side by side
        return np.ascontiguousarray(
            woT.reshape(2, 128, D).transpose(1, 0, 2).reshape(128, 2 * D)
        )

    in_maps = []
    for c in range(NCORES):
        b = c // GROUPS
        g = c % GROUPS
        fs, fe = g * FL, (g + 1) * FL
        in_maps.append({
            "xT": np.ascontiguousarray(x[b].T).astype(bf),
            "wq": pack_dxf(Wq[fs:fe, :].T.astype(bf)),
            "wk": pack_dxf(Wk[fs:fe, :].T.astype(bf)),
            "wv": pack_dxf(Wv[fs:fe, :].T.astype(bf)),
            "wo": pack_fxe(Wo[:, fs:fe].T.astype(bf)),
            "bqc": np.ascontiguousarray(bq[fs:fe].reshape(2, 128).T),
            "bkc": np.ascontiguousarray(bk[fs:fe].reshape(2, 128).T),
            "bvb": np.tile(bv[fs:fe], (128, 1)).astype(np.float32),
            "mb": np.ascontiguousarray(
                np.where(mask[b] == 0, np.float32(-1e9), np.float32(0.0))
                .astype(np.float32).reshape(N_JC, 128).T
            ),
        })
    return in_maps


def kernel(x, mask, Wq, bq, Wk, bk, Wv, bv, Wo, bo):
    bo = np.asarray(bo, dtype=np.float32)
    nc = _get_nc()
    in_maps = make_in_maps(dict(x=x, mask=mask, Wq=Wq, bq=bq, Wk=Wk, bk=bk,
                                Wv=Wv, bv=bv, Wo=Wo, bo=bo))
    res = run_bass_kernel_spmd(nc, in_maps, core_ids=list(range(NCORES)))
    parts = [np.asarray(r["out"], dtype=np.float32) for r in res.results]
    full = np.empty((B, S, D), dtype=np.float32)
    for b in range(B):
        acc = parts[b * GROUPS].copy()
        for g in range(1, GROUPS):
            acc += parts[b * GROUPS + g]
        full[b] = acc + bo[None, :]
    return full
